# revision 15
# baseline (speedup 1.0000x reference)
"""Trainium2 Bass kernel for the DCN (modulated deformable conv) layer.

Self-contained: hardcodes all shapes. Shards data-parallel over (batch b x
row-half h) onto 8 NeuronCores; each core computes a [64, 64, 128] slab of
the [4, 64, 128, 128] output.

Per-core algorithm (all position indices x live on SBUF partitions):
  1. om-conv (3x3, PE, bf16): offsets dy,dx + mask logits from `inter`.
  2. PE-transpose om -> omT[x, y, ch]; build bilinear/sigmoid mask planes
     m[p, sy, sx, y][x] on DVE (positions-in-partitions layout).
  3. PE-transpose feat -> featT5[x, dx_shift, row, c] (5 column shifts via
     DMA partition-shifted copies).
  4. Apply: valT[x, p, y, c] += m * featT5[...]  -- 5184 fused
     scalar_tensor_tensor ops on DVE (mask value is a per-partition scalar).
  5. PE-transpose valT back to val[(c,p)-partitions, (y,x)] and contract with
     the per-sample dynamic weights W2' = c2_w @ (weight * fvec) on PE.
"""

import numpy as np
from contextlib import ExitStack

import concourse.bass as bass
import concourse.bacc as bacc
import concourse.tile as tile
from concourse import mybir
from concourse.bass_utils import run_bass_kernel_spmd

F32 = mybir.dt.float32
BF16 = mybir.dt.bfloat16
ALU = mybir.AluOpType
ACTF = mybir.ActivationFunctionType

B, CIN, COUT, H, W, K = 4, 64, 64, 128, 128, 3
KK = K * K
NOUT = 64          # out rows per core
NR = 69            # feat rows resident per core:  y + ky-1 + sy in [-3, 66)
NRI = 66           # inter rows resident (conv halo 1)
NCH = 16           # position chunks (of 512) per core
CLAMP = 0.999999

_CACHED = {}


def _build_nc():
    nc = bacc.Bacc("TRN2", target_bir_lowering=False)

    # ---- DRAM I/O (per-core views; same program on all 8 cores) ----
    d_feat = nc.dram_tensor("feat", [CIN, NR, 132], F32, kind="ExternalInput")
    d_inter = nc.dram_tensor("inter", [CIN, NRI, 130], F32, kind="ExternalInput")
    d_w2 = nc.dram_tensor("w2", [128, 5, 64], F32, kind="ExternalInput")
    d_comw = nc.dram_tensor("comw", [CIN, KK, 27], F32, kind="ExternalInput")
    d_comb = nc.dram_tensor("comb", [27, 1], F32, kind="ExternalInput")
    d_c1w = nc.dram_tensor("c1w", [128, 2, COUT], F32, kind="ExternalInput")
    d_fea = nc.dram_tensor("fea", [128, 2], F32, kind="ExternalInput")
    d_bias2 = nc.dram_tensor("bias2", [COUT, 1], F32, kind="ExternalInput")
    d_ident = nc.dram_tensor("ident", [128, 128], F32, kind="ExternalInput")
    d_out = nc.dram_tensor("out", [COUT, NOUT, W], F32, kind="ExternalOutput")

    with ExitStack() as ctx:
        tc = ctx.enter_context(tile.TileContext(nc))

        # ---------------- persistent small pool ----------------
        pers = ctx.enter_context(tc.tile_pool(name="pers", bufs=1))
        identb = pers.tile([128, 128], BF16)
        w2cp = pers.tile([128, 5 * 64], F32)
        w2b = pers.tile([128, 5, 64], BF16)
        comwb = pers.tile([CIN, KK, 27], BF16)
        combc = pers.tile([27, 1], F32)
        c1wb = pers.tile([128, 2, COUT], BF16)
        feab = pers.tile([128, 2], BF16)
        bias2c = pers.tile([COUT, 1], F32)
        fvec = pers.tile([128, 1], F32)
        omT = pers.tile([128, NOUT, 27], BF16)
        masks = pers.tile([128, KK, 3, 3, NOUT], F32)

        identf = pers.tile([128, 128], F32)
        nc.sync.dma_start(out=identf, in_=d_ident[:, :])
        nc.vector.tensor_copy(identb[:, :], identf[:, :])
        nc.sync.dma_start(out=w2cp, in_=d_w2.rearrange("p a b -> p (a b)")[:, :])
        dma_comw = nc.gpsimd.dma_start(out=comwb, in_=d_comw[:, :, :])  # cast f32->bf16
        nc.sync.dma_start(out=combc, in_=d_comb[:, :])
        nc.gpsimd.dma_start(out=c1wb, in_=d_c1w[:, :, :])
        nc.gpsimd.dma_start(out=feab, in_=d_fea[:, :])
        nc.sync.dma_start(out=bias2c, in_=d_bias2[:, :])

        psum_sm = ctx.enter_context(tc.tile_pool(name="psum_sm", bufs=1, space="PSUM"))

        # fvec = c1_w @ fea  -> [64, 1]; replicate to [128, 1]
        ps_fv = psum_sm.tile([COUT, 1], F32)
        for k in range(2):
            nc.tensor.matmul(ps_fv[:, :], c1wb[:, k, :], feab[:, k : k + 1],
                             start=(k == 0), stop=(k == 1))
        nc.scalar.copy(fvec[0:COUT, :], ps_fv[:, :])
        nc.sync.dma_start(out=fvec[COUT:128, :], in_=fvec[0:COUT, :])
        # w2b = (w2cp * fvec[c]) cast to bf16
        nc.vector.tensor_scalar(out=w2b.rearrange("p a b -> p (a b)")[:, :],
                                in0=w2cp[:, :], scalar1=fvec[:, :], scalar2=None,
                                op0=ALU.mult)

        # ---------------- phase 1: om conv + masks ----------------
        with tc.tile_pool(name="omph", bufs=1) as omph:
            interb = omph.tile([CIN, NRI, 130], BF16)
            nc.gpsimd.dma_start(out=interb, in_=d_inter[:, :, :])
            om_sb = omph.tile([27, NOUT, W], BF16)
            with tc.tile_pool(name="ompsum", bufs=2, space="PSUM") as ompsum:
                for n in range(NCH):  # 512-wide position chunks = 4 out rows
                    ps = ompsum.tile([27, 512], F32)
                    y0 = 4 * n
                    for d in range(KK):
                        dy, dx = d // 3, d % 3
                        rhs = interb[:, y0 + dy : y0 + dy + 4, dx : dx + W]
                        nc.tensor.matmul(ps[:, :], comwb[:, d, :], rhs,
                                         start=(d == 0), stop=(d == KK - 1))
                    nc.scalar.activation(om_sb[:, y0 : y0 + 4, :].rearrange("p a b -> p (a b)"),
                                         ps[:, :], ACTF.Identity, bias=combc[:, :])
                # om transpose: [27, 128] row-slices -> omT [128, y, 27]
                for g in range(4):  # 16 rows per psum tile (28-elem slots, 4B-aligned)
                    pst = ompsum.tile([128, 16, 28], BF16)
                    for j in range(16):
                        y = 16 * g + j
                        nc.tensor.transpose(pst[:, j, 0:27],
                                            om_sb[:, y, :], identb[0:27, 0:27])
                    nc.scalar.copy(omT[:, 16 * g : 16 * (g + 1), :], pst[:, :, 0:27])

            # ---- mask build (f32), positions on partitions ----
            mbig = omph.tile([128, 8, KK, NOUT], F32)
            dyT, dxT, sgT, ey, ly, ay, f0, s = [mbig[:, i] for i in range(8)]
            wm_t = omph.tile([128, KK, NOUT], F32, tag="wm")
            wm = wm_t[:, :, :]
            w0_t = omph.tile([128, KK, NOUT], F32, tag="w0")
            w0 = w0_t[:, :, :]
            wp_t = omph.tile([128, KK, NOUT], F32, tag="wp")
            wp = wp_t[:, :, :]
            wys = omph.tile([128, KK, 3, NOUT], F32)
            wxs = omph.tile([128, KK, 3, NOUT], F32)
            # repack dy/dx/sig from omT (ch-minor) into [128, p, y] contiguous
            for dst, lo in [(dyT, 0), (dxT, 9), (sgT, 18)]:
                nc.vector.tensor_copy(dst[:, :, :],
                                      omT[:, :, lo : lo + 9].rearrange("p y c -> p c y"))
            nc.scalar.activation(sgT, sgT, ACTF.Sigmoid)

            for dT, wtile, fold_sig in [(dyT, wys, True), (dxT, wxs, False)]:
                nc.vector.tensor_scalar(out=dT[:, :, :], in0=dT[:, :, :], scalar1=-CLAMP,
                                        scalar2=CLAMP, op0=ALU.max, op1=ALU.min)
                nc.vector.tensor_scalar(out=ey, in0=dT[:, :, :], scalar1=0.0,
                                        scalar2=None, op0=ALU.is_lt)
                nc.vector.tensor_tensor(out=ly, in0=dT[:, :, :], in1=ey,
                                        op=ALU.add)
                nc.vector.tensor_scalar(out=ay, in0=ly, scalar1=-1.0,
                                        scalar2=1.0, op0=ALU.mult, op1=ALU.add)
                nc.vector.tensor_scalar(out=f0, in0=ey, scalar1=-1.0,
                                        scalar2=1.0, op0=ALU.mult, op1=ALU.add)
                nc.vector.tensor_tensor(out=wm, in0=ey, in1=ay,
                                        op=ALU.mult)
                nc.vector.tensor_tensor(out=wp, in0=f0, in1=ly,
                                        op=ALU.mult)
                nc.vector.tensor_tensor(out=s, in0=wm, in1=wp,
                                        op=ALU.add)
                nc.vector.tensor_scalar(out=w0, in0=s, scalar1=-1.0,
                                        scalar2=1.0, op0=ALU.mult, op1=ALU.add)
                for k, wk in enumerate([wm, w0, wp]):
                    if fold_sig:
                        nc.vector.tensor_tensor(out=wtile[:, :, k, :], in0=wk[:, :, :],
                                                in1=sgT, op=ALU.mult)
                    else:
                        nc.vector.tensor_copy(wtile[:, :, k, :], wk[:, :, :])
            # m[p, sy, sx, y] = wys[p, sy, y] * wxs[p, sx, y]
            nc.vector.tensor_tensor(
                out=masks[:, :, :, :, :],
                in0=wys[:, :, :, None, :].broadcast_to([128, KK, 3, 3, NOUT]),
                in1=wxs[:, :, None, :, :].broadcast_to([128, KK, 3, 3, NOUT]),
                op=ALU.mult)

        # ---------------- phase 2: featT5 ----------------
        featT5 = pers.tile([128, 5, NR, CIN], BF16)
        nc.vector.memset(featT5[:, :, :, :], 0.0)
        with tc.tile_pool(name="featph", bufs=1) as featph:
            featb = featph.tile([CIN, NR, 132], BF16)
            nc.gpsimd.dma_start(out=featb, in_=d_feat[:, :, :])
            with tc.tile_pool(name="ftpsum", bufs=4, space="PSUM") as ftpsum:
                for g in range((NR + 7) // 8):  # 8 rows per psum tile
                    rows = range(8 * g, min(8 * g + 8, NR))
                    pst = ftpsum.tile([128, 8 * CIN], BF16)
                    for j, r in enumerate(rows):
                        nc.tensor.transpose(pst[:, CIN * j : CIN * (j + 1)],
                                            featb[:, r, 2 : 2 + 128], identb[0:CIN, 0:CIN])
                    nc.scalar.copy(
                        featT5[:, 2, 8 * g : 8 * g + len(rows), :].rearrange("p a b -> p (a b)"),
                        pst[:, : len(rows) * CIN])
        # shifted copies: featT5[:, 2+d, :, :][x] = featT0[x + d]
        flat = featT5.rearrange("p a b c -> p a (b c)")
        for dlt, di in [(-2, 0), (-1, 1), (1, 3), (2, 4)]:
            if dlt > 0:
                nc.sync.dma_start(out=flat[0 : 128 - dlt, di, :], in_=flat[dlt:128, 2, :])
            else:
                nc.sync.dma_start(out=flat[-dlt : 128, di, :], in_=flat[0 : 128 + dlt, 2, :])

        # ---------------- phase 3: apply + back-transpose + einsum ----------------
        with (
            tc.tile_pool(name="vpool", bufs=3) as vpool,
            tc.tile_pool(name="vblk", bufs=2) as vblk,
            tc.tile_pool(name="och", bufs=2) as och,
            tc.tile_pool(name="vpsum", bufs=3, space="PSUM") as vpsum,
        ):
            for nb in range(NCH):  # 4 out rows per block
                vt = vpool.tile([128, 4, KK, CIN], BF16, tag="vt")
                nc.gpsimd.memset(vt[:, :, :, :], 0.0)
                for p in range(KK):
                    ky, kx = p // 3, p % 3
                    for j in range(4):
                        y = 4 * nb + j
                        for sy in range(3):
                            r = y + ky + sy + 1
                            for sx in range(3):
                                di = kx + sx
                                nc.vector.scalar_tensor_tensor(
                                    out=vt[:, j, p, :], in0=featT5[:, di, r, :],
                                    scalar=masks[:, p, sy, sx, y : y + 1],
                                    in1=vt[:, j, p, :], op0=ALU.mult, op1=ALU.add)
                val_blk = vblk.tile([128, 5, 4, W], BF16, tag="vb")
                nc.vector.memset(val_blk[64:128, 4, :, :], 0.0)
                for t in range(5):
                    pst = vpsum.tile([128, 512], BF16, tag="bt")
                    for pp in range(2):
                        p = 2 * t + pp
                        if p >= KK:
                            continue
                        for j in range(4):
                            nc.tensor.transpose(
                                pst[64 * pp : 64 * pp + 64, 128 * j : 128 * (j + 1)],
                                vt[:, j, p, :], identb[:, :])
                    hi = 128 if t < 4 else 64
                    nc.scalar.copy(val_blk[0:hi, t, :, :], pst[0:hi, :])
                ps = vpsum.tile([COUT, 512], F32, tag="mm")
                for t in range(5):
                    nc.tensor.matmul(ps[:, :], w2b[:, t, :], val_blk[:, t, :, :],
                                     start=(t == 0), stop=(t == 4))
                oc = och.tile([COUT, 4, W], F32, tag="oc")
                nc.scalar.activation(oc[:, :, :], ps[:, :], ACTF.Identity,
                                     bias=bias2c[:, :])
                nc.sync.dma_start(out=d_out[:, 4 * nb : 4 * nb + 4, :], in_=oc[:, :, :])

    nc.compile()
    return nc


def _host_prep(inputs):
    """Build the 8 per-core input maps (numpy marshalling only)."""
    feat = np.ascontiguousarray(inputs["input_feat"], dtype=np.float32)
    inter = np.ascontiguousarray(inputs["inter"], dtype=np.float32)
    fea = np.asarray(inputs["fea"], dtype=np.float32)[:, :, 0, 0]  # [B, 256]
    weight = np.asarray(inputs["weight"], dtype=np.float32)
    bias = np.asarray(inputs["bias"], dtype=np.float32)
    com_w = np.asarray(inputs["com_w"], dtype=np.float32)
    com_b = np.asarray(inputs["com_b"], dtype=np.float32)
    c1_w = np.asarray(inputs["c1_w"], dtype=np.float32)
    c2_w = np.asarray(inputs["c2_w"], dtype=np.float32)

    # fold c2 into the static weight:  weight2[o2, c, p] (parameter prep)
    w_r = weight.reshape(COUT, CIN, KK)
    weight2 = np.einsum("ao,ocp->acp", c2_w, w_r)  # [64, 64, 9]
    w2 = np.zeros((128, 5, 64), np.float32)  # [(c, p-pair), ktile, o2]
    for p in range(KK):
        t, pp = p // 2, p % 2
        w2[64 * pp : 64 * pp + 64, t, :] = weight2[:, :, p].T  # [c, o2]
    bias2 = (c2_w @ bias).reshape(COUT, 1)

    # com_w reordered: channels [dy x9, dx x9, sig x9]; layout [cin, tap, 27]
    perm = list(range(0, 18, 2)) + list(range(1, 18, 2)) + list(range(18, 27))
    comw = np.ascontiguousarray(
        com_w[perm].reshape(27, CIN, KK).transpose(1, 2, 0))  # [CIN, KK, 27]
    comb = com_b[perm].reshape(27, 1).astype(np.float32)

    c1w = np.ascontiguousarray(c1_w.T.reshape(2, 128, COUT).transpose(1, 0, 2))
    ident = np.eye(128, dtype=np.float32)

    in_maps = []
    for i in range(8):
        b, h = i // 2, i % 2
        r0 = NOUT * h
        fpad = np.zeros((CIN, NR, 132), np.float32)
        glo, ghi = r0 - 3, r0 - 3 + NR
        slo, shi = max(0, glo), min(H, ghi)
        fpad[:, slo - glo : shi - glo, 2 : 2 + W] = feat[b, :, slo:shi, :]
        ipad = np.zeros((CIN, NRI, 130), np.float32)
        glo, ghi = r0 - 1, r0 - 1 + NRI
        slo, shi = max(0, glo), min(H, ghi)
        ipad[:, slo - glo : shi - glo, 1 : 1 + W] = inter[b, :, slo:shi, :]
        feac = np.ascontiguousarray(fea[b].reshape(2, 128).T)
        in_maps.append(dict(feat=fpad, inter=ipad, w2=w2, comw=comw, comb=comb,
                            c1w=c1w, fea=feac, bias2=bias2, ident=ident))
    return in_maps


def kernel(**inputs) -> np.ndarray:
    if "nc" not in _CACHED:
        _CACHED["nc"] = _build_nc()
    nc = _CACHED["nc"]
    in_maps = _host_prep(inputs)
    res = run_bass_kernel_spmd(nc, in_maps, core_ids=list(range(8)),
                               **_CACHED.get("run_kwargs", {}))
    _CACHED["last_result"] = res
    out = np.zeros((B, COUT, H, W), np.float32)
    for i in range(8):
        b, h = i // 2, i % 2
        out[b, :, NOUT * h : NOUT * (h + 1), :] = res.results[i]["out"]
    return out
